# revision 30
# baseline (speedup 1.0000x reference)
"""Trainium2 Bass kernel for nn_Attention_82403242541756 (v2.2).

Reference semantics (with the dim-0 chunk bug):
  qkv = inputs @ W_qkv + b_qkv                  # [3, 2048, 3072]
  q, k, v = split(qkv, 3, axis=0)               # batch split! q=batch0, k=batch1, v=batch2
  each chunk [1, 2048, 3072] flat-reinterpreted to (3, 16, 2048, 64) = 48 "heads"
  scores softmax (no max subtraction needed; |scores| < 2.2), ctx, flat-reinterpret,
  @ W_out + b_out

Sharding (zero communication): core c takes seq rows [256c, 256c+256) of all 3
batch items -> exactly heads [6c, 6c+6), each a full [2048, 64] attention unit.

Per head l:
  - q/k transposed readback (XBAR), cast to fp8e4 (x8 prescale), scores via
    DoubleRow fp8 matmuls (0.5 cyc/row; the two k-tiles both read the same
    data via stride-0 dim, so result = 2*k.T@q; folded into the exp scale).
  - exp on ACT into expT [w, u] bf16 (the ~210us wall the rest hides under).
  - ctx matmul TRANSPOSED: chains out[u-block 128, 64] = expT-slice.T @ v,
    memset + start=False so 8 chains share one psum bank; softmax denominators
    via 1-col ones matmuls; normalize = one reciprocal + broadcast multiply.
  - normalized ctx [u, d] PE-transposed back to [d, u] tiles (identity
    matmul, psum tiles riding the ctx chain banks) -> ctxn_all [64, 96, 128];
    out-proj reads it with a strided AP so psum cols land in output-row order.
  - QKV emitted in (b, m) blocks ordered so heads 0-2 (m=0 of b0/b1) finish
    first; frontends emitted ahead of backends to keep ACT saturated; the
    last head's backend is split in halves to shrink the serial tail.
"""

import sys

sys.path.insert(0, "/opt/trn_rl_repo")

import contextlib

import numpy as np
import ml_dtypes

from concourse import bacc, bass, mybir, tile
from concourse.bass_utils import run_bass_kernel_spmd

BF16 = mybir.dt.bfloat16
F32 = mybir.dt.float32
FP8 = mybir.dt.float8e4
AF = mybir.ActivationFunctionType
ALU = mybir.AluOpType
DR = mybir.MatmulPerfMode.DoubleRow

P = 128
N_CORES = 8
SEQ = 2048
H = 1024
HEADS = 6
ROWS = 256  # seq rows per core
QSCALE = 8.0  # fp8 prescale on q and k (avoids subnormals)
# scores psum = 2 * (8q).(8k) => fold 1/(2*64) on top of H**-0.5
SCALE = float(H) ** -0.5 / (2.0 * QSCALE * QSCALE)

_NC_CACHE = {}


def _build():
    nc = bacc.Bacc()

    xt_e = nc.declare_dram_parameter("xt", [P, 8, 768], BF16, isOutput=False)
    wq_e = nc.declare_dram_parameter("wq", [P, 8, 3072], BF16, isOutput=False)
    bq_e = nc.declare_dram_parameter("bq", [P, 3072], BF16, isOutput=False)
    wo_e = nc.declare_dram_parameter("wo", [64, 16, 1024], BF16, isOutput=False)
    bo_e = nc.declare_dram_parameter("bo", [P, 8], F32, isOutput=False)
    id_e = nc.declare_dram_parameter("ident", [P, P], BF16, isOutput=False)
    out_e = nc.declare_dram_parameter("outt", [1024, 768], F32, isOutput=True)

    with tile.TileContext(nc) as tc:
        with (
            tc.tile_pool(name="dram", bufs=1, space="DRAM") as dp,
            tc.tile_pool(name="qk", bufs=5) as qkp,
            tc.tile_pool(name="qk8", bufs=4) as q8p,
            tc.tile_pool(name="vex", bufs=2) as vxp,
            tc.tile_pool(name="scps", bufs=2, space="PSUM") as scp,
            tc.tile_pool(name="expp", bufs=3) as expp,
            tc.tile_pool(name="idp", bufs=1) as idp,
        ):
            # Padded to 128 cols so the bf16 XBAR DMA-transpose readback is legal.
            yq = dp.tile([12288, 128], BF16)
            yk = dp.tile([12288, 128], BF16)
            yv = dp.tile([12288, 64], BF16)
            yq_v = yq.rearrange("(r j) d -> r j d", j=48)
            yk_v = yk.rearrange("(r j) d -> r j d", j=48)
            yv_v = yv.rearrange("(r j) d -> r (j d)", j=48)

            zpad = idp.tile([P, 64], BF16)
            nc.vector.memset(zpad[:], 0.0)
            ident = idp.tile([P, P], BF16)

            # ---------------- Phase 1 pools (closed after the b=2 block) ----
            es = contextlib.ExitStack()
            w1p = es.enter_context(tc.tile_pool(name="w1", bufs=1))
            ps1 = es.enter_context(tc.tile_pool(name="ps1", bufs=4, space="PSUM"))
            ybp = es.enter_context(tc.tile_pool(name="yb", bufs=4))

            xt_sb = w1p.tile([P, 8, 768], BF16)
            # consumption order: b0 m0, b1 m0 first, then the rest
            nc.gpsimd.dma_start(xt_sb[:, :, 0:128], xt_e[:, :, 0:128])
            nc.gpsimd.dma_start(xt_sb[:, :, 256:384], xt_e[:, :, 256:384])
            wq_sb = w1p.tile([P, 8, 3072], BF16)
            bq_sb = w1p.tile([P, 3072], BF16)
            # stream W_qkv in consumption order across 3 queues; bq rides the
            # scalar queue early (first bias-add needs it at ~9us)
            wq_engs = (nc.sync, nc.gpsimd, nc.scalar)
            for half in range(2):
                for k in range(8):
                    wq_engs[k % 3].dma_start(
                        wq_sb[:, k, 1536 * half : 1536 * (half + 1)],
                        wq_e[:, k, 1536 * half : 1536 * (half + 1)],
                    )
                if half == 0:
                    nc.scalar.dma_start(bq_sb[:], bq_e[:])
            nc.gpsimd.dma_start(xt_sb[:, :, 128:256], xt_e[:, :, 128:256])
            nc.gpsimd.dma_start(xt_sb[:, :, 384:768], xt_e[:, :, 384:768])
            # one-time zero fill of the yq/yk DRAM pad cols (XBAR transpose
            # reads 128-col multiples; partitions 64:128 are never consumed
            # but CoreSim requires defined data). Per-head chunks: heads 0-1
            # early (gate the first transposes), the rest late and spread.
            zsrc16 = zpad.rearrange("p (o c) -> p o c", o=1).to_broadcast(
                [P, 16, 64]
            )

            def pad_fill(y, l, eng):
                eng.dma_start(
                    y.rearrange("(a p) c -> p a c", p=P)[
                        :, 16 * l : 16 * (l + 1), 64:128
                    ],
                    zsrc16,
                )

            for l in range(2):
                pad_fill(yq, l, nc.scalar)
                pad_fill(yk, l, nc.scalar)
            for l in range(2, 6):
                pad_fill(yq, l, nc.gpsimd)
                pad_fill(yk, l, nc.gpsimd)
            nc.gpsimd.dma_start(ident[:], id_e[:])

            def emit_qkv_block(b, m, r0=0, nr=128):
                for half in range(2):
                    psums = {}
                    for nb3 in range(3):
                        psums[3 * half + nb3] = ps1.tile(
                            [P, 512], F32, name=f"yps{3*half+nb3}", tag="yps"
                        )
                    for k in range(8):
                        c0 = b * 256 + 128 * m + r0
                        lhs = xt_sb[:, k, c0 : c0 + nr]
                        for nb3 in range(3):
                            nb = 3 * half + nb3
                            nc.tensor.matmul(
                                psums[nb][0:nr, :],
                                lhsT=lhs,
                                rhs=wq_sb[:, k, 512 * nb : 512 * (nb + 1)],
                                start=(k == 0),
                                stop=(k == 7),
                            )
                    emit_qkv_bias(b, m, r0, nr, half, psums)

            def emit_qkv_bias(b, m, r0, nr, half, psums):
                for nb in range(3 * half, 3 * half + 3):
                    if b < 2:
                        # only the 64 valid cols; yq/yk pad cols stay garbage
                        # (transpose readback partitions 64:128 are never read)
                        ybuf = ybp.tile([P, 8, 64], BF16, tag="ybw")
                        nc.vector.tensor_tensor(
                            ybuf[0:nr, :, :],
                            psums[nb][0:nr].rearrange("p (j d) -> p j d", d=64),
                            bq_sb[:, 512 * nb : 512 * (nb + 1)].rearrange(
                                "p (j d) -> p j d", d=64
                            )[0:nr],
                            ALU.add,
                        )
                        r1 = 128 * m + r0
                        dst = (yq_v if b == 0 else yk_v)[
                            r1 : r1 + nr, 8 * nb : 8 * (nb + 1), 0:64
                        ]
                        nc.sync.dma_start(dst, ybuf[0:nr])
                    else:
                        ybuf = ybp.tile([P, 512], BF16, tag="ybn")
                        nc.vector.tensor_tensor(
                            ybuf[0:nr],
                            psums[nb][0:nr],
                            bq_sb[:, 512 * nb : 512 * (nb + 1)][0:nr],
                            ALU.add,
                        )
                        nc.sync.dma_start(
                            yv_v[
                                128 * m + r0 : 128 * m + r0 + nr,
                                512 * nb : 512 * (nb + 1),
                            ],
                            ybuf[0:nr],
                        )

            def emit_vx(l):
                # emitted AFTER the b=2 qkv blocks (Tile orders by emission)
                vx = vxp.tile([P, 16, 65], BF16, name=f"vx{l}", tag="vx")
                nc.vector.memset(vx[:, :, 64:65], 1.0)
                nc.sync.dma_start(
                    vx[:, :, 0:64],
                    yv[SEQ * l : SEQ * (l + 1), :].rearrange("(so p) d -> p so d", p=P),
                )
                return vx

            def emit_frontend(l):
                qT = qkp.tile([P, SEQ], BF16, tag="qk", name=f"qT{l}")
                nc.sync.dma_start(qT[:], yq[SEQ * l : SEQ * (l + 1), :], transpose=True)
                kT = qkp.tile([P, SEQ], BF16, tag="qk", name=f"kT{l}")
                keng = nc.scalar if l == 0 else nc.sync
                keng.dma_start(kT[:], yk[SEQ * l : SEQ * (l + 1), :], transpose=True)
                # cast to fp8 with x8 prescale
                qT8 = q8p.tile([64, SEQ], FP8, tag="q8", name=f"qT8{l}")
                nc.vector.tensor_scalar(
                    qT8[:, 0:1024], qT[0:64, 0:1024], QSCALE, None, ALU.mult
                )
                nc.gpsimd.tensor_scalar(
                    qT8[:, 1024:2048], qT[0:64, 1024:2048], QSCALE, None, ALU.mult
                )
                kT8 = q8p.tile([64, SEQ], FP8, tag="q8", name=f"kT8{l}")
                nc.vector.tensor_scalar(
                    kT8[:, 0:256], kT[0:64, 0:256], QSCALE, None, ALU.mult
                )
                nc.gpsimd.tensor_scalar(
                    kT8[:, 256:2048], kT[0:64, 256:2048], QSCALE, None, ALU.mult
                )
                expTs = []
                for th in range(2):
                    expTs.append(
                        expp.tile([P, 8, SEQ], BF16, tag="expT", name=f"expT{l}_{th}")
                    )
                # last head: uh-major so the u<1024 backend half can fully
                # drain (chains, norm, transposes, out-proj) before the final
                # exps of the u>=1024 half land
                if l == HEADS - 1:
                    order = [(tt, uh) for uh in range(2) for tt in range(16)]
                else:
                    order = [(tt, uh) for tt in range(16) for uh in range(2)]
                for tt, uh in order:
                    th, t8 = divmod(tt, 8)
                    lhsT = (
                        kT8[:, 128 * tt : 128 * (tt + 1)]
                        .rearrange("p (o n) -> p o n", o=1)
                        .to_broadcast([64, 2, 128])
                    )
                    sc = scp.tile([P, 1024], F32, name=f"sc{l}_{tt}_{uh}", tag="sc")
                    for s2 in range(2):
                        u0 = 1024 * uh + 512 * s2
                        nc.tensor.matmul(
                            sc[:, 512 * s2 : 512 * (s2 + 1)],
                            lhsT=lhsT,
                            rhs=qT8[:, u0 : u0 + 512]
                            .rearrange("p (o n) -> p o n", o=1)
                            .to_broadcast([64, 2, 512]),
                            start=True,
                            stop=True,
                            perf_mode=DR,
                        )
                    nc.scalar.activation(
                        expTs[th][:, t8, 1024 * uh : 1024 * (uh + 1)],
                        sc[:],
                        AF.Exp,
                        scale=SCALE,
                    )
                return expTs

            emit_qkv_block(0, 0)
            emit_qkv_block(1, 0)
            h0_expTs = emit_frontend(0)
            emit_qkv_block(0, 1)
            emit_qkv_block(1, 1)
            h1_expTs = emit_frontend(1)
            emit_qkv_block(2, 0)
            emit_qkv_block(2, 1)
            es.close()  # release w1/ps1/yb space

            # ---------------- Phase 2: attention backend + out-proj ---------
            with (
                tc.tile_pool(name="w2", bufs=1) as w2p,
                tc.tile_pool(name="cnl", bufs=2) as cnp,
                tc.tile_pool(name="rr", bufs=2) as rrp,
                tc.tile_pool(name="stg", bufs=2) as stgp,
                tc.tile_pool(name="ctx1", bufs=1, space="PSUM") as cx1,
                tc.tile_pool(name="ctx2", bufs=1, space="PSUM") as cx2,
                tc.tile_pool(name="tpd", bufs=1, space="PSUM") as tpp,
                tc.tile_pool(name="ops", bufs=1, space="PSUM") as opp,
            ):
                wo_sb = w2p.tile([64, 16, 1024], BF16)
                nc.gpsimd.dma_start(wo_sb[:], wo_e[:])
                bo_sb = w2p.tile([P, 8], F32)
                nc.gpsimd.dma_start(bo_sb[:], bo_e[:])
                # transposed normalized context: [d, (l j), uu]
                ctxn_all = w2p.tile([64, 96, 128], BF16)
                cxp = (cx1, cx2)

                def emit_backend(l, vx, expTs):
                    # ctx chains tt-major: the tt-slice of all 16 u-block
                    # chains runs as soon as exp(tt) lands, so only the last
                    # slice trails the head's final exp. memset + start=False
                    # lets 8 chains share each psum bank.
                    den = tpp.tile([P, 16], F32, name=f"den{l}", tag="tpd")
                    nc.vector.memset(den[:], 0.0)
                    tiles = []
                    views = []
                    for g in range(2):
                        t = cxp[g].tile([P, 512], F32, name=f"ctx{l}_{g}", tag="c")
                        nc.vector.memset(t[:], 0.0)
                        tiles.append(t)
                        views.append(t.rearrange("p (j c) -> p j c", c=64))
                    ones = vx[:, 0, 64:65]
                    if l == HEADS - 1:
                        jorder = [(tt, j) for g in range(2) for tt in range(16)
                                  for j in range(8 * g, 8 * g + 8)]
                    else:
                        jorder = [(tt, j) for tt in range(16) for j in range(16)]
                    for tt, j in jorder:
                        th, t8 = divmod(tt, 8)
                        g, jj = divmod(j, 8)
                        lhsT = expTs[th][:, t8, 128 * j : 128 * (j + 1)]
                        nc.tensor.matmul(
                            views[g][:, jj, :],
                            lhsT=lhsT,
                            rhs=vx[:, tt, 0:64],
                            start=False,
                            stop=(tt == 15),
                            skip_group_check=True,
                        )
                        nc.tensor.matmul(
                            den[:, j : j + 1],
                            lhsT=lhsT,
                            rhs=ones,
                            start=False,
                            stop=(tt == 15),
                            skip_group_check=True,
                        )
                    # normalize per bank-half: reciprocal + broadcast mult,
                    # then transpose back to [d, uu]; two psum lanes. On the
                    # final head the copies alternate DVE/ACT to drain faster.
                    rr = rrp.tile([P, 16], F32, tag="rr")
                    ctxn_h = cnp.tile([P, 16, 64], BF16, tag="cn")
                    final = l == HEADS - 1
                    for g in range(2):
                        nc.vector.reciprocal(
                            rr[:, 8 * g : 8 * (g + 1)], den[:, 8 * g : 8 * (g + 1)]
                        )
                        nc.vector.tensor_tensor(
                            ctxn_h[:, 8 * g : 8 * (g + 1), :],
                            views[g][:],
                            rr[:, 8 * g : 8 * (g + 1)]
                            .rearrange("p (j o) -> p j o", o=1)
                            .to_broadcast([P, 8, 64]),
                            ALU.mult,
                        )
                        for jj in range(8):
                            j = 8 * g + jj
                            tp_t = cxp[j % 2].tile([64, P], BF16, tag="c")
                            nc.tensor.transpose(tp_t[:], ctxn_h[:, j, :], ident[:])
                            if final and j % 2 == 1:
                                nc.scalar.activation(
                                    ctxn_all[:, 16 * l + j, :],
                                    tp_t[:],
                                    AF.Copy,
                                    scale=1.0,
                                )
                            else:
                                nc.vector.tensor_copy(
                                    out=ctxn_all[:, 16 * l + j, :], in_=tp_t[:]
                                )

                out_v = out_e.rearrange("(mm p) c -> p mm c", p=P)

                def emit_outproj(lj0, lj1, tail=False):
                    # cols lj0*8 .. lj1*8 of the per-core output rows; m-blocks
                    # staged into one tile, written out in two half DMAs. Tail
                    # chunks also rotate through the (idle by then) sc psum
                    # pool to avoid a PE<->DVE ping-pong on a single psum buf.
                    n = 8 * (lj1 - lj0)
                    rhs4 = ctxn_all[:, lj0:lj1, :].rearrange(
                        "d l (r t) -> d t l r", t=16
                    )
                    stg = stgp.tile([P, 8, n], F32, tag="stg")
                    pools = (opp, scp) if tail else (opp,)
                    for m in range(8):
                        pool = pools[m % len(pools)]
                        ops = pool.tile([P, n], F32, tag="op" if pool is opp else "sc")
                        for tp in range(16):
                            nc.tensor.matmul(
                                ops[:],
                                lhsT=wo_sb[:, tp, 128 * m : 128 * (m + 1)],
                                rhs=rhs4[:, tp, :, :],
                                start=(tp == 0),
                                stop=(tp == 15),
                            )
                        nc.vector.tensor_scalar(
                            stg[:, m, :], ops[:], bo_sb[:, m : m + 1], None, ALU.add
                        )
                        if m == 3:
                            nc.gpsimd.dma_start(
                                out_v[:, 0:4, 8 * lj0 : 8 * lj1], stg[:, 0:4, :]
                            )
                    nc.gpsimd.dma_start(
                        out_v[:, 4:8, 8 * lj0 : 8 * lj1], stg[:, 4:8, :]
                    )

                exp_tiles = {0: h0_expTs, 1: h1_expTs}
                exp_tiles[2] = emit_frontend(2)
                emit_backend(0, emit_vx(0), exp_tiles[0])
                exp_tiles[3] = emit_frontend(3)
                emit_backend(1, emit_vx(1), exp_tiles[1])
                emit_outproj(0, 24)
                exp_tiles[4] = emit_frontend(4)
                emit_backend(2, emit_vx(2), exp_tiles[2])
                emit_outproj(24, 48)
                exp_tiles[5] = emit_frontend(5)
                emit_backend(3, emit_vx(3), exp_tiles[3])
                emit_backend(4, emit_vx(4), exp_tiles[4])
                emit_outproj(48, 72)
                emit_outproj(72, 80)
                emit_backend(5, emit_vx(5), exp_tiles[5])
                emit_outproj(80, 88, tail=True)
                emit_outproj(88, 92, tail=True)
                emit_outproj(92, 96, tail=True)

    nc.finalize()
    return nc


def _get_nc():
    if "nc" not in _NC_CACHE:
        _NC_CACHE["nc"] = _build()
    return _NC_CACHE["nc"]


def kernel(inputs, W_qkv, b_qkv, W_out, b_out, _trace=False, _trace_kwargs=None):
    bf = ml_dtypes.bfloat16
    x = np.asarray(inputs, dtype=np.float32)
    Wq = np.asarray(W_qkv, dtype=np.float32)
    bq = np.asarray(b_qkv, dtype=np.float32)
    Wo = np.asarray(W_out, dtype=np.float32)
    bo = np.asarray(b_out, dtype=np.float32)

    wq_s = np.ascontiguousarray(Wq.reshape(8, P, 3072).transpose(1, 0, 2)).astype(bf)
    wo_s = np.ascontiguousarray(Wo.reshape(16, 64, 1024).transpose(1, 0, 2)).astype(bf)
    bq_s = np.ascontiguousarray(np.broadcast_to(bq[None, :], (P, 3072))).astype(bf)
    bo_s = np.ascontiguousarray(bo.reshape(8, P).T).astype(np.float32)
    id_s = np.eye(P, dtype=bf)

    in_maps = []
    for c in range(N_CORES):
        xc = x[:, ROWS * c : ROWS * (c + 1), :]  # [3, 256, 1024]
        xt = (
            xc.transpose(2, 0, 1)
            .reshape(1024, 768)
            .reshape(8, P, 768)
            .transpose(1, 0, 2)
        )
        in_maps.append(
            {
                "xt": np.ascontiguousarray(xt).astype(bf),
                "wq": wq_s,
                "bq": bq_s,
                "wo": wo_s,
                "bo": bo_s,
                "ident": id_s,
            }
        )

    nc = _get_nc()
    kw = {}
    if _trace:
        kw["trace"] = True
        if _trace_kwargs:
            kw.update(_trace_kwargs)
    res = run_bass_kernel_spmd(nc, in_maps, core_ids=list(range(N_CORES)), **kw)
    outs = res.results

    out = np.empty((6144, 1024), dtype=np.float32)
    for c in range(N_CORES):
        out[768 * c : 768 * (c + 1), :] = np.asarray(
            outs[c]["outt"], dtype=np.float32
        ).T
    if _trace:
        kernel.last_result = res
    return out.reshape(3, SEQ, H)


# revision 31
# speedup vs baseline: 1.0098x; 1.0098x over previous
"""Trainium2 Bass kernel for nn_Attention_82403242541756 (v2.2).

Reference semantics (with the dim-0 chunk bug):
  qkv = inputs @ W_qkv + b_qkv                  # [3, 2048, 3072]
  q, k, v = split(qkv, 3, axis=0)               # batch split! q=batch0, k=batch1, v=batch2
  each chunk [1, 2048, 3072] flat-reinterpreted to (3, 16, 2048, 64) = 48 "heads"
  scores softmax (no max subtraction needed; |scores| < 2.2), ctx, flat-reinterpret,
  @ W_out + b_out

Sharding (zero communication): core c takes seq rows [256c, 256c+256) of all 3
batch items -> exactly heads [6c, 6c+6), each a full [2048, 64] attention unit.

Per head l:
  - q/k transposed readback (XBAR), cast to fp8e4 (x8 prescale), scores via
    DoubleRow fp8 matmuls (0.5 cyc/row; the two k-tiles both read the same
    data via stride-0 dim, so result = 2*k.T@q; folded into the exp scale).
  - exp on ACT into expT [w, u] bf16 (the ~210us wall the rest hides under).
  - ctx matmul TRANSPOSED: chains out[u-block 128, 64] = expT-slice.T @ v,
    memset + start=False so 8 chains share one psum bank; softmax denominators
    via 1-col ones matmuls; normalize = one reciprocal + broadcast multiply.
  - normalized ctx [u, d] PE-transposed back to [d, u] tiles (identity
    matmul, psum tiles riding the ctx chain banks) -> ctxn_all [64, 96, 128];
    out-proj reads it with a strided AP so psum cols land in output-row order.
  - QKV emitted in (b, m) blocks ordered so heads 0-2 (m=0 of b0/b1) finish
    first; frontends emitted ahead of backends to keep ACT saturated; the
    last head's backend is split in halves to shrink the serial tail.
"""

import sys

sys.path.insert(0, "/opt/trn_rl_repo")

import contextlib

import numpy as np
import ml_dtypes

from concourse import bacc, bass, mybir, tile
from concourse.bass_utils import run_bass_kernel_spmd

BF16 = mybir.dt.bfloat16
F32 = mybir.dt.float32
FP8 = mybir.dt.float8e4
AF = mybir.ActivationFunctionType
ALU = mybir.AluOpType
DR = mybir.MatmulPerfMode.DoubleRow

P = 128
N_CORES = 8
SEQ = 2048
H = 1024
HEADS = 6
ROWS = 256  # seq rows per core
QSCALE = 8.0  # fp8 prescale on q and k (avoids subnormals)
# scores psum = 2 * (8q).(8k) => fold 1/(2*64) on top of H**-0.5
SCALE = float(H) ** -0.5 / (2.0 * QSCALE * QSCALE)

_NC_CACHE = {}


def _build():
    nc = bacc.Bacc()

    xt_e = nc.declare_dram_parameter("xt", [P, 8, 768], BF16, isOutput=False)
    wq_e = nc.declare_dram_parameter("wq", [P, 8, 3072], BF16, isOutput=False)
    bq_e = nc.declare_dram_parameter("bq", [P, 3072], BF16, isOutput=False)
    wo_e = nc.declare_dram_parameter("wo", [64, 16, 1024], BF16, isOutput=False)
    bo_e = nc.declare_dram_parameter("bo", [P, 8], F32, isOutput=False)
    id_e = nc.declare_dram_parameter("ident", [P, P], BF16, isOutput=False)
    out_e = nc.declare_dram_parameter("outt", [1024, 768], F32, isOutput=True)

    with tile.TileContext(nc) as tc:
        with (
            tc.tile_pool(name="dram", bufs=1, space="DRAM") as dp,
            tc.tile_pool(name="qk", bufs=5) as qkp,
            tc.tile_pool(name="qk8", bufs=4) as q8p,
            tc.tile_pool(name="vex", bufs=2) as vxp,
            tc.tile_pool(name="scps", bufs=2, space="PSUM") as scp,
            tc.tile_pool(name="expp", bufs=3) as expp,
            tc.tile_pool(name="idp", bufs=1) as idp,
        ):
            # Padded to 128 cols so the bf16 XBAR DMA-transpose readback is legal.
            yq = dp.tile([12288, 128], BF16)
            yk = dp.tile([12288, 128], BF16)
            yv = dp.tile([12288, 64], BF16)
            yq_v = yq.rearrange("(r j) d -> r j d", j=48)
            yk_v = yk.rearrange("(r j) d -> r j d", j=48)
            yv_v = yv.rearrange("(r j) d -> r (j d)", j=48)

            zpad = idp.tile([P, 64], BF16)
            nc.vector.memset(zpad[:], 0.0)
            ident = idp.tile([P, P], BF16)

            # ---------------- Phase 1 pools (closed after the b=2 block) ----
            es = contextlib.ExitStack()
            w1p = es.enter_context(tc.tile_pool(name="w1", bufs=1))
            ps1 = es.enter_context(tc.tile_pool(name="ps1", bufs=4, space="PSUM"))
            ybp = es.enter_context(tc.tile_pool(name="yb", bufs=4))

            xt_sb = w1p.tile([P, 8, 768], BF16)
            # consumption order: b0 m0, b1 m0 first, then the rest
            nc.gpsimd.dma_start(xt_sb[:, :, 0:128], xt_e[:, :, 0:128])
            nc.gpsimd.dma_start(xt_sb[:, :, 256:384], xt_e[:, :, 256:384])
            wq_sb = w1p.tile([P, 8, 3072], BF16)
            bq_sb = w1p.tile([P, 3072], BF16)
            # stream W_qkv in consumption order across 3 queues; bq rides the
            # scalar queue early (first bias-add needs it at ~9us)
            wq_engs = (nc.sync, nc.gpsimd, nc.scalar)
            for half in range(2):
                for k in range(8):
                    wq_engs[k % 3].dma_start(
                        wq_sb[:, k, 1536 * half : 1536 * (half + 1)],
                        wq_e[:, k, 1536 * half : 1536 * (half + 1)],
                    )
                if half == 0:
                    nc.scalar.dma_start(bq_sb[:], bq_e[:])
            nc.gpsimd.dma_start(xt_sb[:, :, 128:256], xt_e[:, :, 128:256])
            nc.gpsimd.dma_start(xt_sb[:, :, 384:768], xt_e[:, :, 384:768])
            # one-time zero fill of the yq/yk DRAM pad cols (XBAR transpose
            # reads 128-col multiples; partitions 64:128 are never consumed
            # but CoreSim requires defined data). Per-head chunks: heads 0-1
            # early (gate the first transposes), the rest late and spread.
            zsrc = zpad.rearrange("p (o c) -> p o c", o=1).to_broadcast([P, 96, 64])
            nc.scalar.dma_start(
                yq.rearrange("(a p) c -> p a c", p=P)[:, :, 64:128], zsrc
            )
            nc.gpsimd.dma_start(
                yk.rearrange("(a p) c -> p a c", p=P)[:, :, 64:128], zsrc
            )
            nc.gpsimd.dma_start(ident[:], id_e[:])

            def emit_qkv_block(b, m, r0=0, nr=128):
                for half in range(2):
                    psums = {}
                    for nb3 in range(3):
                        psums[3 * half + nb3] = ps1.tile(
                            [P, 512], F32, name=f"yps{3*half+nb3}", tag="yps"
                        )
                    for k in range(8):
                        c0 = b * 256 + 128 * m + r0
                        lhs = xt_sb[:, k, c0 : c0 + nr]
                        for nb3 in range(3):
                            nb = 3 * half + nb3
                            nc.tensor.matmul(
                                psums[nb][0:nr, :],
                                lhsT=lhs,
                                rhs=wq_sb[:, k, 512 * nb : 512 * (nb + 1)],
                                start=(k == 0),
                                stop=(k == 7),
                            )
                    emit_qkv_bias(b, m, r0, nr, half, psums)

            def emit_qkv_bias(b, m, r0, nr, half, psums):
                for nb in range(3 * half, 3 * half + 3):
                    if b < 2:
                        # only the 64 valid cols; yq/yk pad cols stay garbage
                        # (transpose readback partitions 64:128 are never read)
                        ybuf = ybp.tile([P, 8, 64], BF16, tag="ybw")
                        nc.vector.tensor_tensor(
                            ybuf[0:nr, :, :],
                            psums[nb][0:nr].rearrange("p (j d) -> p j d", d=64),
                            bq_sb[:, 512 * nb : 512 * (nb + 1)].rearrange(
                                "p (j d) -> p j d", d=64
                            )[0:nr],
                            ALU.add,
                        )
                        r1 = 128 * m + r0
                        dst = (yq_v if b == 0 else yk_v)[
                            r1 : r1 + nr, 8 * nb : 8 * (nb + 1), 0:64
                        ]
                        nc.sync.dma_start(dst, ybuf[0:nr])
                    else:
                        ybuf = ybp.tile([P, 512], BF16, tag="ybn")
                        nc.vector.tensor_tensor(
                            ybuf[0:nr],
                            psums[nb][0:nr],
                            bq_sb[:, 512 * nb : 512 * (nb + 1)][0:nr],
                            ALU.add,
                        )
                        nc.sync.dma_start(
                            yv_v[
                                128 * m + r0 : 128 * m + r0 + nr,
                                512 * nb : 512 * (nb + 1),
                            ],
                            ybuf[0:nr],
                        )

            def emit_vx(l):
                # emitted AFTER the b=2 qkv blocks (Tile orders by emission)
                vx = vxp.tile([P, 16, 65], BF16, name=f"vx{l}", tag="vx")
                nc.vector.memset(vx[:, :, 64:65], 1.0)
                nc.sync.dma_start(
                    vx[:, :, 0:64],
                    yv[SEQ * l : SEQ * (l + 1), :].rearrange("(so p) d -> p so d", p=P),
                )
                return vx

            def emit_frontend(l):
                qT = qkp.tile([P, SEQ], BF16, tag="qk", name=f"qT{l}")
                nc.sync.dma_start(qT[:], yq[SEQ * l : SEQ * (l + 1), :], transpose=True)
                kT = qkp.tile([P, SEQ], BF16, tag="qk", name=f"kT{l}")
                keng = nc.scalar if l == 0 else nc.sync
                keng.dma_start(kT[:], yk[SEQ * l : SEQ * (l + 1), :], transpose=True)
                # cast to fp8 with x8 prescale
                qT8 = q8p.tile([64, SEQ], FP8, tag="q8", name=f"qT8{l}")
                nc.vector.tensor_scalar(
                    qT8[:, 0:1024], qT[0:64, 0:1024], QSCALE, None, ALU.mult
                )
                nc.gpsimd.tensor_scalar(
                    qT8[:, 1024:2048], qT[0:64, 1024:2048], QSCALE, None, ALU.mult
                )
                kT8 = q8p.tile([64, SEQ], FP8, tag="q8", name=f"kT8{l}")
                nc.vector.tensor_scalar(
                    kT8[:, 0:256], kT[0:64, 0:256], QSCALE, None, ALU.mult
                )
                nc.gpsimd.tensor_scalar(
                    kT8[:, 256:2048], kT[0:64, 256:2048], QSCALE, None, ALU.mult
                )
                expTs = []
                for th in range(2):
                    expTs.append(
                        expp.tile([P, 8, SEQ], BF16, tag="expT", name=f"expT{l}_{th}")
                    )
                # last head: uh-major so the u<1024 backend half can fully
                # drain (chains, norm, transposes, out-proj) before the final
                # exps of the u>=1024 half land
                if l == HEADS - 1:
                    order = [(tt, uh) for uh in range(2) for tt in range(16)]
                else:
                    order = [(tt, uh) for tt in range(16) for uh in range(2)]
                for tt, uh in order:
                    th, t8 = divmod(tt, 8)
                    lhsT = (
                        kT8[:, 128 * tt : 128 * (tt + 1)]
                        .rearrange("p (o n) -> p o n", o=1)
                        .to_broadcast([64, 2, 128])
                    )
                    sc = scp.tile([P, 1024], F32, name=f"sc{l}_{tt}_{uh}", tag="sc")
                    for s2 in range(2):
                        u0 = 1024 * uh + 512 * s2
                        nc.tensor.matmul(
                            sc[:, 512 * s2 : 512 * (s2 + 1)],
                            lhsT=lhsT,
                            rhs=qT8[:, u0 : u0 + 512]
                            .rearrange("p (o n) -> p o n", o=1)
                            .to_broadcast([64, 2, 512]),
                            start=True,
                            stop=True,
                            perf_mode=DR,
                        )
                    nc.scalar.activation(
                        expTs[th][:, t8, 1024 * uh : 1024 * (uh + 1)],
                        sc[:],
                        AF.Exp,
                        scale=SCALE,
                    )
                return expTs

            emit_qkv_block(0, 0)
            emit_qkv_block(1, 0)
            h0_expTs = emit_frontend(0)
            emit_qkv_block(0, 1)
            emit_qkv_block(1, 1)
            h1_expTs = emit_frontend(1)
            emit_qkv_block(2, 0)
            emit_qkv_block(2, 1)
            es.close()  # release w1/ps1/yb space

            # ---------------- Phase 2: attention backend + out-proj ---------
            with (
                tc.tile_pool(name="w2", bufs=1) as w2p,
                tc.tile_pool(name="cnl", bufs=2) as cnp,
                tc.tile_pool(name="rr", bufs=2) as rrp,
                tc.tile_pool(name="stg", bufs=2) as stgp,
                tc.tile_pool(name="ctx1", bufs=1, space="PSUM") as cx1,
                tc.tile_pool(name="ctx2", bufs=1, space="PSUM") as cx2,
                tc.tile_pool(name="tpd", bufs=1, space="PSUM") as tpp,
                tc.tile_pool(name="ops", bufs=1, space="PSUM") as opp,
            ):
                wo_sb = w2p.tile([64, 16, 1024], BF16)
                nc.gpsimd.dma_start(wo_sb[:], wo_e[:])
                bo_sb = w2p.tile([P, 8], F32)
                nc.gpsimd.dma_start(bo_sb[:], bo_e[:])
                # transposed normalized context: [d, (l j), uu]
                ctxn_all = w2p.tile([64, 96, 128], BF16)
                cxp = (cx1, cx2)

                def emit_backend(l, vx, expTs):
                    # ctx chains tt-major: the tt-slice of all 16 u-block
                    # chains runs as soon as exp(tt) lands, so only the last
                    # slice trails the head's final exp. memset + start=False
                    # lets 8 chains share each psum bank.
                    den = tpp.tile([P, 16], F32, name=f"den{l}", tag="tpd")
                    nc.vector.memset(den[:], 0.0)
                    tiles = []
                    views = []
                    for g in range(2):
                        t = cxp[g].tile([P, 512], F32, name=f"ctx{l}_{g}", tag="c")
                        nc.vector.memset(t[:], 0.0)
                        tiles.append(t)
                        views.append(t.rearrange("p (j c) -> p j c", c=64))
                    ones = vx[:, 0, 64:65]
                    if l == HEADS - 1:
                        jorder = [(tt, j) for g in range(2) for tt in range(16)
                                  for j in range(8 * g, 8 * g + 8)]
                    else:
                        jorder = [(tt, j) for tt in range(16) for j in range(16)]
                    for tt, j in jorder:
                        th, t8 = divmod(tt, 8)
                        g, jj = divmod(j, 8)
                        lhsT = expTs[th][:, t8, 128 * j : 128 * (j + 1)]
                        nc.tensor.matmul(
                            views[g][:, jj, :],
                            lhsT=lhsT,
                            rhs=vx[:, tt, 0:64],
                            start=False,
                            stop=(tt == 15),
                            skip_group_check=True,
                        )
                        nc.tensor.matmul(
                            den[:, j : j + 1],
                            lhsT=lhsT,
                            rhs=ones,
                            start=False,
                            stop=(tt == 15),
                            skip_group_check=True,
                        )
                    # normalize per bank-half: reciprocal + broadcast mult,
                    # then transpose back to [d, uu]; two psum lanes. On the
                    # final head the copies alternate DVE/ACT to drain faster.
                    rr = rrp.tile([P, 16], F32, tag="rr")
                    ctxn_h = cnp.tile([P, 16, 64], BF16, tag="cn")
                    final = l == HEADS - 1
                    for g in range(2):
                        nc.vector.reciprocal(
                            rr[:, 8 * g : 8 * (g + 1)], den[:, 8 * g : 8 * (g + 1)]
                        )
                        nc.vector.tensor_tensor(
                            ctxn_h[:, 8 * g : 8 * (g + 1), :],
                            views[g][:],
                            rr[:, 8 * g : 8 * (g + 1)]
                            .rearrange("p (j o) -> p j o", o=1)
                            .to_broadcast([P, 8, 64]),
                            ALU.mult,
                        )
                        for jj in range(8):
                            j = 8 * g + jj
                            tp_t = cxp[j % 2].tile([64, P], BF16, tag="c")
                            nc.tensor.transpose(tp_t[:], ctxn_h[:, j, :], ident[:])
                            if final and j % 2 == 1:
                                nc.scalar.activation(
                                    ctxn_all[:, 16 * l + j, :],
                                    tp_t[:],
                                    AF.Copy,
                                    scale=1.0,
                                )
                            else:
                                nc.vector.tensor_copy(
                                    out=ctxn_all[:, 16 * l + j, :], in_=tp_t[:]
                                )

                out_v = out_e.rearrange("(mm p) c -> p mm c", p=P)

                def emit_outproj(lj0, lj1, tail=False):
                    # cols lj0*8 .. lj1*8 of the per-core output rows; m-blocks
                    # staged into one tile, written out in two half DMAs. Tail
                    # chunks also rotate through the (idle by then) sc psum
                    # pool to avoid a PE<->DVE ping-pong on a single psum buf.
                    n = 8 * (lj1 - lj0)
                    rhs4 = ctxn_all[:, lj0:lj1, :].rearrange(
                        "d l (r t) -> d t l r", t=16
                    )
                    stg = stgp.tile([P, 8, n], F32, tag="stg")
                    pools = (opp, scp) if tail else (opp,)
                    for m in range(8):
                        pool = pools[m % len(pools)]
                        ops = pool.tile([P, n], F32, tag="op" if pool is opp else "sc")
                        for tp in range(16):
                            nc.tensor.matmul(
                                ops[:],
                                lhsT=wo_sb[:, tp, 128 * m : 128 * (m + 1)],
                                rhs=rhs4[:, tp, :, :],
                                start=(tp == 0),
                                stop=(tp == 15),
                            )
                        nc.vector.tensor_scalar(
                            stg[:, m, :], ops[:], bo_sb[:, m : m + 1], None, ALU.add
                        )
                        if m == 3:
                            nc.gpsimd.dma_start(
                                out_v[:, 0:4, 8 * lj0 : 8 * lj1], stg[:, 0:4, :]
                            )
                    nc.gpsimd.dma_start(
                        out_v[:, 4:8, 8 * lj0 : 8 * lj1], stg[:, 4:8, :]
                    )

                exp_tiles = {0: h0_expTs, 1: h1_expTs}
                exp_tiles[2] = emit_frontend(2)
                emit_backend(0, emit_vx(0), exp_tiles[0])
                exp_tiles[3] = emit_frontend(3)
                emit_backend(1, emit_vx(1), exp_tiles[1])
                emit_outproj(0, 24)
                exp_tiles[4] = emit_frontend(4)
                emit_backend(2, emit_vx(2), exp_tiles[2])
                emit_outproj(24, 48)
                exp_tiles[5] = emit_frontend(5)
                emit_backend(3, emit_vx(3), exp_tiles[3])
                emit_backend(4, emit_vx(4), exp_tiles[4])
                emit_outproj(48, 72)
                emit_outproj(72, 80)
                emit_backend(5, emit_vx(5), exp_tiles[5])
                emit_outproj(80, 88, tail=True)
                emit_outproj(88, 92, tail=True)
                emit_outproj(92, 96, tail=True)

    nc.finalize()
    return nc


def _get_nc():
    if "nc" not in _NC_CACHE:
        _NC_CACHE["nc"] = _build()
    return _NC_CACHE["nc"]


def kernel(inputs, W_qkv, b_qkv, W_out, b_out, _trace=False, _trace_kwargs=None):
    bf = ml_dtypes.bfloat16
    x = np.asarray(inputs, dtype=np.float32)
    Wq = np.asarray(W_qkv, dtype=np.float32)
    bq = np.asarray(b_qkv, dtype=np.float32)
    Wo = np.asarray(W_out, dtype=np.float32)
    bo = np.asarray(b_out, dtype=np.float32)

    wq_s = np.ascontiguousarray(Wq.reshape(8, P, 3072).transpose(1, 0, 2)).astype(bf)
    wo_s = np.ascontiguousarray(Wo.reshape(16, 64, 1024).transpose(1, 0, 2)).astype(bf)
    bq_s = np.ascontiguousarray(np.broadcast_to(bq[None, :], (P, 3072))).astype(bf)
    bo_s = np.ascontiguousarray(bo.reshape(8, P).T).astype(np.float32)
    id_s = np.eye(P, dtype=bf)

    in_maps = []
    for c in range(N_CORES):
        xc = x[:, ROWS * c : ROWS * (c + 1), :]  # [3, 256, 1024]
        xt = (
            xc.transpose(2, 0, 1)
            .reshape(1024, 768)
            .reshape(8, P, 768)
            .transpose(1, 0, 2)
        )
        in_maps.append(
            {
                "xt": np.ascontiguousarray(xt).astype(bf),
                "wq": wq_s,
                "bq": bq_s,
                "wo": wo_s,
                "bo": bo_s,
                "ident": id_s,
            }
        )

    nc = _get_nc()
    kw = {}
    if _trace:
        kw["trace"] = True
        if _trace_kwargs:
            kw.update(_trace_kwargs)
    res = run_bass_kernel_spmd(nc, in_maps, core_ids=list(range(N_CORES)), **kw)
    outs = res.results

    out = np.empty((6144, 1024), dtype=np.float32)
    for c in range(N_CORES):
        out[768 * c : 768 * (c + 1), :] = np.asarray(
            outs[c]["outt"], dtype=np.float32
        ).T
    if _trace:
        kernel.last_result = res
    return out.reshape(3, SEQ, H)
